# revision 62
# baseline (speedup 1.0000x reference)
import numpy as np

NV = 100000
NTOT = 200000
C = 2048
CPC = 256            # clusters per core
NCORES = 8
CHUNKS = 196         # output chunks of 128 ids per core
IDS_PER_CORE = CHUNKS * 128          # 25088 (also the x shard size)
TPAD = NCORES * IDS_PER_CORE         # 200704 padded id space
SEND_REAL = CPC * 128                # 32768 h rows per core
SEND_ROWS = SEND_REAL + 128          # + zero block
GAMMA = 1.0
SCALE = 8.0          # sqrt(64)

_cache = {}
_mesh_cache = {}
_pool = None


def _get_pool():
    global _pool
    if _pool is None:
        from concurrent.futures import ThreadPoolExecutor
        _pool = ThreadPoolExecutor(8)
    return _pool


def _par_rows(n, nch, fn):
    """run fn(lo, hi) over nch row-chunks of [0, n) in threads"""
    pool = _get_pool()
    bnds = [(i * n // nch, (i + 1) * n // nch) for i in range(nch)]
    list(pool.map(lambda b: fn(*b), bnds))


def _build(BPC, CHUNKS_P):
    import concourse.bass as bass
    import concourse.mybir as mybir
    import concourse.tile as tile
    import concourse.bacc as bacc
    from concourse.masks import make_identity

    f32 = mybir.dt.float32
    f16 = mybir.dt.float16
    i32 = mybir.dt.int32
    i8 = mybir.dt.int8
    NBLK = CHUNKS_P * BPC
    IDS_P = CHUNKS_P * 128
    TPAD_P = NCORES * IDS_P

    u16 = mybir.dt.uint16
    u8 = mybir.dt.uint8

    nc = bacc.Bacc("TRN2", target_bir_lowering=False, debug=False)
    xq8 = nc.dram_tensor("xq8", [IDS_P, 64], i8, kind="ExternalInput")
    aux16 = nc.dram_tensor("aux16", [64, CPC], f16, kind="ExternalInput")
    xg_lo = nc.dram_tensor("xg_lo", [128, CPC], u16, kind="ExternalInput")
    xg_hi = nc.dram_tensor("xg_hi", [128, CPC], u8, kind="ExternalInput")
    mrg_lo = nc.dram_tensor("mrg_lo", [128, NBLK], u16, kind="ExternalInput")
    mrg_hi = nc.dram_tensor("mrg_hi", [128, NBLK], u8, kind="ExternalInput")
    ids8 = nc.dram_tensor("ids8", [128, NBLK], i8, kind="ExternalInput")
    wts = nc.dram_tensor("wts", [65, 192], f32, kind="ExternalInput")
    out_p = nc.dram_tensor("out_p", [IDS_P, 48], u8, kind="ExternalOutput")
    out_s = nc.dram_tensor("out_s", [IDS_P, 1], f16, kind="ExternalOutput")

    xloc8 = nc.dram_tensor("xloc8", [IDS_P, 64], i8)
    ptab8 = nc.dram_tensor("ptab8", [TPAD_P, 64], i8, addr_space="Shared")
    send = nc.dram_tensor("send", [SEND_ROWS, 64], f32)
    allh = nc.dram_tensor("allh", [NCORES * SEND_ROWS, 64], f32, addr_space="Shared")

    with tile.TileContext(nc) as tc:
        # materialize the full quantized node table on every core
        nc.sync.dma_start(out=xloc8[:], in_=xq8[:])
        nc.gpsimd.collective_compute(
            "AllGather", mybir.AluOpType.bypass,
            replica_groups=[list(range(NCORES))],
            ins=[xloc8[:]], outs=[ptab8[:]])

        with tc.tile_pool(name="const", bufs=1) as cp:
            ident = cp.tile([128, 128], f32)
            make_identity(nc, ident[:])
            iot_i = cp.tile([128, 128], i32)
            nc.gpsimd.iota(out=iot_i[:], pattern=[[1, 128]], base=0, channel_multiplier=0)
            iot_f = cp.tile([128, 128], f32)
            nc.vector.tensor_copy(out=iot_f[:], in_=iot_i[:])
            wts_sb = cp.tile([65, 192], f32)
            nc.sync.dma_start(out=wts_sb[:], in_=wts[:])
            bt_sb = wts_sb[0:64, 0:64]
            wv_sb = wts_sb[0:64, 64:128]
            wo_sb = wts_sb[0:65, 128:192]
            aux_sb = cp.tile([64, CPC], f16)
            nc.sync.dma_start(out=aux_sb[:], in_=aux16[:])
            aux_f = cp.tile([128, CPC], f32)
            nc.gpsimd.memset(aux_f[0:64, :], 0.0)
            nc.vector.tensor_copy(out=aux_f[64:128, :], in_=aux_sb[:])
            def unpack24(lo_t, hi_t, ncols, out_tile):
                lo_sb = cp.tile([128, ncols], u16)
                nc.sync.dma_start(out=lo_sb[:], in_=lo_t[:])
                hi_sb = cp.tile([128, ncols], u8)
                nc.sync.dma_start(out=hi_sb[:], in_=hi_t[:])
                lo32 = cp.tile([128, ncols], i32)
                nc.vector.tensor_copy(out=lo32[:], in_=lo_sb[:])
                hi32 = cp.tile([128, ncols], i32)
                nc.vector.tensor_copy(out=hi32[:], in_=hi_sb[:])
                his = cp.tile([128, ncols], i32)
                nc.vector.tensor_scalar(out=his[:], in0=hi32[:], scalar1=16,
                                        scalar2=None,
                                        op0=mybir.AluOpType.logical_shift_left)
                nc.vector.tensor_tensor(out=out_tile[:], in0=lo32[:], in1=his[:],
                                        op=mybir.AluOpType.add)

            xo_sb = cp.tile([128, CPC], i32)
            unpack24(xg_lo, xg_hi, CPC, xo_sb)
            mo_sb = cp.tile([128, NBLK], i32)
            unpack24(mrg_lo, mrg_hi, NBLK, mo_sb)
            id8_sb = cp.tile([128, NBLK], i8)
            nc.sync.dma_start(out=id8_sb[:], in_=ids8[:])
            id_sb = cp.tile([128, NBLK], f32)
            nc.vector.tensor_copy(out=id_sb[:], in_=id8_sb[:])

            # ---------- phase A: per-cluster attention ----------
            with tc.tile_pool(name="asb", bufs=3) as asb, \
                 tc.tile_pool(name="aps", bufs=1, space="PSUM") as aps, \
                 tc.tile_pool(name="aps2", bufs=2, space="PSUM") as aps2, \
                 tc.tile_pool(name="xt4p", bufs=2) as xt4p, \
                 tc.tile_pool(name="xgp", bufs=6) as xgp:
                for g in range(CPC // 4):
                    XT4 = xt4p.tile([64, 512], f32)
                    for c4 in range(4):
                        c = g * 4 + c4
                        xg8 = xgp.tile([128, 64], i8, tag="xg")
                        nc.gpsimd.indirect_dma_start(
                            out=xg8[:, :], out_offset=None, in_=ptab8[:],
                            in_offset=bass.IndirectOffsetOnAxis(ap=xo_sb[:, c:c + 1], axis=0))
                        xgf = xgp.tile([128, 64], f32, tag="xgf")
                        nc.vector.tensor_copy(out=xgf[:], in_=xg8[:])
                        tp = aps.tile([64, 128], f32, tag="tp")
                        nc.tensor.transpose(out=tp[:], in_=xgf[:], identity=ident[:])
                        nc.any.tensor_copy(out=XT4[:, c4 * 128:(c4 + 1) * 128], in_=tp[:])
                    P4p = aps.tile([64, 512], f32, tag="p4")
                    nc.tensor.matmul(out=P4p[:], lhsT=bt_sb, rhs=XT4[:], start=True, stop=True)
                    P4 = asb.tile([64, 512], f32, tag="p4s")
                    nc.any.tensor_copy(out=P4[:], in_=P4p[:])
                    h4 = asb.tile([128, 4, 64], f32, tag="h4")
                    for c4 in range(4):
                        c = g * 4 + c4
                        cs = slice(c4 * 128, (c4 + 1) * 128)
                        Vp = aps.tile([128, 64], f32, tag="vp")
                        nc.tensor.matmul(out=Vp[:], lhsT=XT4[:, cs], rhs=wv_sb, start=True, stop=True)
                        Vx = asb.tile([128, 65], f32, tag="vx")
                        nc.gpsimd.memset(Vx[:, 64:65], 1.0)
                        nc.any.tensor_copy(out=Vx[:, 0:64], in_=Vp[:])
                        STp = aps2.tile([128, 128], f32, tag="st")
                        nc.tensor.matmul(out=STp[:], lhsT=XT4[:, cs], rhs=P4[:, cs], start=True, stop=True)
                        y1 = asb.tile([128, 128], f32, tag="y1")
                        nc.vector.tensor_scalar(out=y1[:], in0=STp[:],
                                                scalar1=aux_f[:, c:c + 1], scalar2=None,
                                                op0=mybir.AluOpType.add)
                        y2 = asb.tile([128, 128], f32, tag="y2")
                        nc.vector.tensor_scalar(out=y2[:], in0=STp[:],
                                                scalar1=aux_f[:, c:c + 1], scalar2=0.2,
                                                op0=mybir.AluOpType.add,
                                                op1=mybir.AluOpType.mult)
                        L = asb.tile([128, 128], f32, tag="lr")
                        nc.vector.tensor_tensor(out=L[:], in0=y1[:], in1=y2[:],
                                                op=mybir.AluOpType.max)
                        E = asb.tile([128, 128], f32, tag="ex")
                        nc.scalar.activation(out=E[:], in_=L[:],
                                             func=mybir.ActivationFunctionType.Exp)
                        Hp = aps2.tile([128, 65], f32, tag="hp")
                        nc.tensor.matmul(out=Hp[:], lhsT=E[:], rhs=Vx[:], start=True, stop=True)
                        rec = asb.tile([128, 1], f32, tag="rec")
                        nc.vector.reciprocal(out=rec[:], in_=Hp[:, 64:65])
                        nc.vector.tensor_scalar_mul(h4[:, c4, :], Hp[:, 0:64], rec[:])
                    nc.sync.dma_start(
                        out=send[g * 512:(g + 1) * 512, :].rearrange("(c p) d -> p c d", p=128),
                        in_=h4[:, :, :])
                zz = asb.tile([128, 64], f32, tag="zz")
                nc.gpsimd.memset(zz[:], 0.0)
                nc.sync.dma_start(out=send[SEND_REAL:SEND_ROWS, :], in_=zz[:])

            # ---------- exchange ----------
            nc.gpsimd.collective_compute(
                "AllGather", mybir.AluOpType.bypass,
                replica_groups=[list(range(NCORES))],
                ins=[send[:]], outs=[allh[:]])

            # ---------- phase B: segment-sum + project + quantize ----------
            with tc.tile_pool(name="bsb", bufs=4) as bsb, \
                 tc.tile_pool(name="bps", bufs=2, space="PSUM") as bps:
                for j in range(CHUNKS_P):
                    stgs = []
                    ohs = []
                    for w in range(BPC):
                        b = j * BPC + w
                        stg = bsb.tile([128, 65], f32, tag="stg")
                        nc.gpsimd.memset(stg[:, 64:65], 1.0)
                        nc.gpsimd.indirect_dma_start(
                            out=stg[:, 0:64], out_offset=None, in_=allh[:],
                            in_offset=bass.IndirectOffsetOnAxis(ap=mo_sb[:, b:b + 1], axis=0))
                        stgs.append(stg)
                        oh = bsb.tile([128, 128], f32, tag="oh")
                        nc.vector.tensor_tensor(out=oh[:], in0=id_sb[:, b:b + 1].to_broadcast([128, 128]),
                                                in1=iot_f[:], op=mybir.AluOpType.is_equal)
                        ohs.append(oh)
                    oT = bps.tile([65, 128], f32, tag="ot")
                    for w in range(BPC):
                        nc.tensor.matmul(out=oT[:], lhsT=stgs[w][:, :], rhs=ohs[w][:],
                                         start=(w == 0), stop=(w == BPC - 1))
                    cnat = bps.tile([128, 1], f32, tag="cn")
                    for w in range(BPC):
                        nc.tensor.matmul(out=cnat[:], lhsT=ohs[w][:], rhs=stgs[w][:, 64:65],
                                         start=(w == 0), stop=(w == BPC - 1))
                    oTs = bsb.tile([65, 128], f32, tag="ots")
                    nc.any.tensor_copy(out=oTs[:], in_=oT[:])
                    cm = bsb.tile([128, 1], f32, tag="cm")
                    nc.vector.tensor_scalar_max(cm[:], cnat[:], 1.0)
                    rc = bsb.tile([128, 1], f32, tag="rc")
                    nc.vector.reciprocal(out=rc[:], in_=cm[:])
                    fp = bps.tile([128, 64], f32, tag="fp")
                    nc.tensor.matmul(out=fp[:], lhsT=oTs[:], rhs=wo_sb, start=True, stop=True)
                    fs = bsb.tile([128, 64], f32, tag="fs")
                    nc.vector.tensor_scalar_mul(fs[:], fp[:], rc[:])
                    # 6-bit row quantization: scale = absmax/31, +32 offset,
                    # column blocks v0..v3 = cols [0:16][16:32][32:48][48:64]
                    # packed into 48 bytes: b0=v0|(v1&3)<<6  b1=(v1>>2)|(v2&15)<<4
                    # b2=(v2>>4)|v3<<2
                    am = bsb.tile([128, 1], f32, tag="am")
                    nc.vector.tensor_reduce(out=am[:], in_=fs[:], axis=mybir.AxisListType.X,
                                            op=mybir.AluOpType.max, apply_absolute_value=True)
                    amc = bsb.tile([128, 1], f32, tag="amc")
                    nc.vector.tensor_scalar_max(amc[:], am[:], 1e-6)
                    s16 = bsb.tile([128, 1], f16, tag="s16")
                    nc.vector.tensor_scalar(out=s16[:], in0=amc[:], scalar1=1.0 / 31.0,
                                            scalar2=None, op0=mybir.AluOpType.mult)
                    r1 = bsb.tile([128, 1], f32, tag="r1")
                    nc.vector.reciprocal(out=r1[:], in_=amc[:])
                    r2 = bsb.tile([128, 1], f32, tag="r2")
                    nc.vector.tensor_scalar(out=r2[:], in0=r1[:], scalar1=31.0,
                                            scalar2=None, op0=mybir.AluOpType.mult)
                    q = bsb.tile([128, 64], f32, tag="q")
                    nc.vector.tensor_scalar(out=q[:], in0=fs[:], scalar1=r2[:],
                                            scalar2=32.0, op0=mybir.AluOpType.mult,
                                            op1=mybir.AluOpType.add)
                    qi = bsb.tile([128, 64], i32, tag="qi")
                    nc.vector.tensor_copy(out=qi[:], in_=q[:])
                    v0, v1, v2, v3 = (qi[:, 16 * t:16 * (t + 1)] for t in range(4))
                    pk = bsb.tile([128, 48], i32, tag="pk")
                    ta = bsb.tile([128, 16], i32, tag="ta")
                    nc.vector.tensor_scalar(out=ta[:], in0=v1, scalar1=3, scalar2=6,
                                            op0=mybir.AluOpType.bitwise_and,
                                            op1=mybir.AluOpType.logical_shift_left)
                    nc.vector.tensor_tensor(out=pk[:, 0:16], in0=v0, in1=ta[:],
                                            op=mybir.AluOpType.bitwise_or)
                    tb = bsb.tile([128, 16], i32, tag="tb")
                    nc.vector.tensor_scalar(out=tb[:], in0=v2, scalar1=15, scalar2=4,
                                            op0=mybir.AluOpType.bitwise_and,
                                            op1=mybir.AluOpType.logical_shift_left)
                    tc = bsb.tile([128, 16], i32, tag="tc")
                    nc.vector.tensor_scalar(out=tc[:], in0=v1, scalar1=2, scalar2=None,
                                            op0=mybir.AluOpType.logical_shift_right)
                    nc.vector.tensor_tensor(out=pk[:, 16:32], in0=tc[:], in1=tb[:],
                                            op=mybir.AluOpType.bitwise_or)
                    td = bsb.tile([128, 16], i32, tag="td")
                    nc.vector.tensor_scalar(out=td[:], in0=v3, scalar1=2, scalar2=None,
                                            op0=mybir.AluOpType.logical_shift_left)
                    te = bsb.tile([128, 16], i32, tag="te")
                    nc.vector.tensor_scalar(out=te[:], in0=v2, scalar1=4, scalar2=None,
                                            op0=mybir.AluOpType.logical_shift_right)
                    nc.vector.tensor_tensor(out=pk[:, 32:48], in0=te[:], in1=td[:],
                                            op=mybir.AluOpType.bitwise_or)
                    pk8 = bsb.tile([128, 48], u8, tag="pk8")
                    nc.vector.tensor_copy(out=pk8[:], in_=pk[:])
                    nc.sync.dma_start(out=out_p[j * 128:(j + 1) * 128, :], in_=pk8[:])
                    nc.sync.dma_start(out=out_s[j * 128:(j + 1) * 128, :], in_=s16[:])

    nc.compile()
    return nc


def _get_mesh():
    if "mesh" not in _mesh_cache:
        import jax
        from jax.sharding import Mesh, PartitionSpec, NamedSharding
        devices = jax.devices()[:NCORES]
        mesh = Mesh(np.asarray(devices), ("core",))
        _mesh_cache["mesh"] = mesh
        _mesh_cache["sh"] = NamedSharding(mesh, PartitionSpec("core"))
    return _mesh_cache["mesh"], _mesh_cache["sh"]


def _make_exec(nc):
    import jax
    import jax.numpy as jnp
    import concourse.mybir as mybir
    from concourse.bass2jax import install_neuronx_cc_hook, partition_id_tensor, _bass_exec_p
    from jax.sharding import PartitionSpec, NamedSharding
    from jax.experimental.shard_map import shard_map

    install_neuronx_cc_hook()
    partition_name = nc.partition_id_tensor.name if nc.partition_id_tensor else None
    in_names, out_names, out_avals = [], [], []
    for alloc in nc.m.functions[0].allocations:
        if not isinstance(alloc, mybir.MemoryLocationSet):
            continue
        name = alloc.memorylocations[0].name
        if alloc.kind == "ExternalInput":
            if name != partition_name:
                in_names.append(name)
        elif alloc.kind == "ExternalOutput":
            out_names.append(name)
            out_avals.append(jax.core.ShapedArray(
                tuple(alloc.tensor_shape), mybir.dt.np(alloc.dtype)))
    n_params = len(in_names)
    n_outs = len(out_names)
    all_names = list(in_names) + list(out_names)
    if partition_name is not None:
        all_names.append(partition_name)

    def _body(*args):
        operands = list(args)
        if partition_name is not None:
            operands.append(partition_id_tensor())
        outs = _bass_exec_p.bind(
            *operands,
            out_avals=tuple(out_avals),
            in_names=tuple(all_names),
            out_names=tuple(out_names),
            lowering_input_output_aliases=(),
            sim_require_finite=True,
            sim_require_nnan=True,
            nc=nc,
        )
        return tuple(outs)

    donate = tuple(range(n_params, n_params + n_outs))
    mesh, sh = _get_mesh()
    spec = sh.spec
    sharded = jax.jit(
        shard_map(_body, mesh=mesh, in_specs=(spec,) * (n_params + n_outs),
                  out_specs=(spec,) * n_outs, check_rep=False),
        donate_argnums=donate, keep_unused=True)
    zshapes = [(NCORES * a.shape[0],) + tuple(a.shape[1:]) for a in out_avals]
    zdtypes = [a.dtype for a in out_avals]
    zeros_fn = jax.jit(
        lambda: tuple(jnp.zeros(s, d) for s, d in zip(zshapes, zdtypes)),
        out_shardings=tuple(NamedSharding(mesh, spec) for _ in out_avals))
    return dict(sharded=sharded, zeros_fn=zeros_fn,
                in_names=in_names, out_names=out_names)


CAP_CHUNKS = 144     # compacted table: 144*128*8 = 147456 unique-node capacity


def run(inputs):
    import jax

    mesh, sh = _get_mesh()
    # warm zero buffers on device while the host preps
    zeros_key = _mesh_cache.get("last_key")
    zeros = _cache[zeros_key][1]["zeros_fn"]() if zeros_key in _cache else None

    x_var = np.asarray(inputs["x_var"], np.float32)
    x_clause = np.asarray(inputs["x_clause"], np.float32)
    sat = np.asarray(inputs["satisfaction_scores"], np.float32)
    cvi = np.asarray(inputs["cluster_var_ids"]).astype(np.int64)
    cci = np.asarray(inputs["cluster_clause_ids"]).astype(np.int64)
    pool = _get_pool()

    # kick the contribution sort off in the background; it gates the big
    # upload, so everything else on the wire goes first
    def _sort_job():
        nodes = np.concatenate([cvi, cci + NV], 1)         # [C, 128]
        flat = nodes.reshape(-1).astype(np.int32)
        order = np.argsort(flat)
        sflat = flat[order]
        newg = np.empty(len(sflat), bool)
        newg[0] = True
        np.not_equal(sflat[1:], sflat[:-1], out=newg[1:])
        uids = sflat[newg]
        crank = np.cumsum(newg) - 1
        cflat = np.empty(len(sflat), np.int32)
        cflat[order] = crank
        return order, uids, crank, cflat

    fsort = pool.submit(_sort_job)

    # aux (bias columns) — ready immediately, fills otherwise-idle wire time
    bias_tab = (GAMMA * sat).astype(np.float16)[cci]       # [C, 64] clause slots
    aux_g = np.empty((NCORES * 64, CPC), np.float16)
    for i in range(NCORES):
        aux_g[i * 64:(i + 1) * 64] = bias_tab[i * CPC:(i + 1) * CPC].T
    dev_aux = jax.device_put(aux_g, sh)

    # global input scale + folded weights
    qn = NV // 4
    gmaxs = list(pool.map(
        lambda a: float(np.abs(a).max()),
        [x_var[i * qn:(i + 1) * qn] for i in range(4)] +
        [x_clause[i * qn:(i + 1) * qn] for i in range(4)]))
    s_in = max(max(gmaxs), 1e-8) / 127.0
    r_in = 1.0 / s_in

    W_Q = np.asarray(inputs["W_Q"], np.float32)
    W_K = np.asarray(inputs["W_K"], np.float32)
    W_V = np.asarray(inputs["W_V"], np.float32)
    hww = np.asarray(inputs["head_weights"], np.float32)
    ah = int(inputs["active_heads"])
    Wo = np.asarray(inputs["out_proj_w"], np.float32)
    bo = np.asarray(inputs["out_proj_b"], np.float32)
    hw = float(np.mean(hww[:ah]))

    B_Tm = (W_Q.T @ W_K / SCALE).astype(np.float32) * (s_in * s_in)
    W_VTm = (W_V * (hw * s_in)).T.copy().astype(np.float32)
    W_oTm = np.vstack([Wo.T, np.zeros((1, 64), np.float32)]).astype(np.float32)
    wts_1 = np.zeros((65, 192), np.float32)
    wts_1[0:64, 0:64] = B_Tm
    wts_1[0:64, 64:128] = W_VTm
    wts_1[0:65, 128:192] = W_oTm
    wts_g = np.tile(wts_1, (NCORES, 1))
    dev_wts = jax.device_put(wts_g, sh)

    order, uids, crank, cflat = fsort.result()
    U = len(uids)
    CHUNKS_P = CAP_CHUNKS if U <= CAP_CHUNKS * 128 * NCORES else CHUNKS
    IDS_P = CHUNKS_P * 128
    TPAD_P = NCORES * IDS_P
    k = int(np.searchsorted(uids, NV))                     # var/clause split in uids

    # gather + global-scale int8 quantize the referenced x rows (threaded)
    xq = np.empty((TPAD_P, 64), np.int8)
    xq[U:] = 0
    uv = uids[:k]
    uc = uids[k:] - NV

    def _q(dst_off, idx, src):
        def work(lo, hi):
            xa = src[idx[lo:hi]]
            xa *= r_in
            np.rint(xa, out=xa)
            xq[dst_off + lo:dst_off + hi] = xa
        _par_rows(len(idx), 8, work)

    _q(0, uv, x_var)
    _q(k, uc, x_clause)
    dev_xq = jax.device_put(xq, sh)                        # starts the big upload

    # gather offsets (overlapped with the upload above)
    cnodes = cflat.reshape(C, 128)                         # compacted ids [C, 128]
    xg_g = np.empty((NCORES * 128, CPC), np.int32)
    for i in range(NCORES):
        xg_g[i * 128:(i + 1) * 128] = cnodes[i * CPC:(i + 1) * CPC].T

    # Output-row permutation: deal ids into blocks by descending contribution
    # count so per-block totals stay near the mean (keeps BPC at 2).
    cnt = np.bincount(crank, minlength=U)
    NB = TPAD_P // 128
    rnk = np.argsort(-cnt)
    ii = np.arange(U)
    orow = np.empty(U, np.int64)
    orow[rnk] = (ii % NB) * 128 + ii // NB

    # merge maps: contributions grouped by output block
    cidx = np.arange(C * 128) // 128
    slot = np.arange(C * 128) % 128
    allh_row = ((cidx // CPC) * SEND_ROWS + (cidx % CPC) * 128 + slot).astype(np.int64)
    srows = allh_row[order].astype(np.int32)   # sorted by compact id
    ZROW = SEND_REAL   # core 0's zero block

    opos = orow[crank]                         # output position per contribution
    blk = opos // 128
    order2 = np.argsort(blk)
    sblk = blk[order2]
    bstart = np.searchsorted(sblk, np.arange(NB))
    rank = np.arange(len(sblk)) - bstart[sblk]
    maxc = int(rank.max()) + 1
    BPC = max(2, -(-maxc // 128))
    S = BPC * 128
    NBLK = CHUNKS_P * BPC

    core = sblk // CHUNKS_P
    jj = sblk % CHUNKS_P
    pos = jj * S + rank
    mrg_full = np.full((NCORES, CHUNKS_P * S), ZROW, np.int32)
    ids_full = np.full((NCORES, CHUNKS_P * S), -1, np.int8)
    mrg_full[core, pos] = srows[order2]
    ids_full[core, pos] = (opos % 128)[order2].astype(np.int8)
    mrg_g = np.ascontiguousarray(
        mrg_full.reshape(NCORES, NBLK, 128).transpose(0, 2, 1)).reshape(NCORES * 128, NBLK)
    ids_g = np.ascontiguousarray(
        ids_full.reshape(NCORES, NBLK, 128).transpose(0, 2, 1)).reshape(NCORES * 128, NBLK)

    key = (BPC, CHUNKS_P)
    if key not in _cache:
        nc = _build(BPC, CHUNKS_P)
        _cache[key] = (nc, _make_exec(nc))
    _mesh_cache["last_key"] = key
    nc, ex = _cache[key]
    if zeros is None or zeros_key != key:
        zeros = ex["zeros_fn"]()

    dev = {"xq8": dev_xq, "aux16": dev_aux, "wts": dev_wts}
    glob = {"xg_lo": (xg_g & 0xFFFF).astype(np.uint16),
            "xg_hi": (xg_g >> 16).astype(np.uint8),
            "mrg_lo": (mrg_g & 0xFFFF).astype(np.uint16),
            "mrg_hi": (mrg_g >> 16).astype(np.uint8),
            "ids8": ids_g}
    args = [dev[n] if n in dev else jax.device_put(glob[n], sh)
            for n in ex["in_names"]]
    out_arrs = ex["sharded"](*args, *zeros)

    omap = {n: a for n, a in zip(ex["out_names"], out_arrs)}
    for a in out_arrs:
        try:
            a.copy_to_host_async()
        except Exception:
            pass
    fq = pool.submit(np.asarray, omap["out_p"])
    fsc = pool.submit(np.asarray, omap["out_s"])

    # overlap the residual base with the output fetch
    outv = np.empty_like(x_var)
    outc = np.empty_like(x_clause)

    def _base(dst, x_src):
        def work(lo, hi):
            np.add(x_src[lo:hi], bo, out=dst[lo:hi])
        _par_rows(x_src.shape[0], 2, work)

    _base(outv, x_var)
    _base(outc, x_clause)
    q_host = fq.result()
    s_host = fsc.result()

    def _scatter(dst, idx, off):
        def work(lo, hi):
            rows = orow[off + lo:off + hi]
            b = q_host[rows].astype(np.int16)
            b0, b1, b2 = b[:, 0:16], b[:, 16:32], b[:, 32:48]
            d = np.empty((len(rows), 64), np.float32)
            d[:, 0:16] = b0 & 63
            d[:, 16:32] = (b0 >> 6) | ((b1 & 15) << 2)
            d[:, 32:48] = (b1 >> 4) | ((b2 & 3) << 4)
            d[:, 48:64] = b2 >> 2
            d -= 32.0
            d *= s_host[rows].astype(np.float32)
            dst[idx[lo:hi]] += d
        _par_rows(len(idx), 4, work)

    _scatter(outv, uv, 0)
    _scatter(outc, uc, k)
    return (outv, outc)


def kernel(**inputs):
    try:
        return run(inputs)
    except Exception:
        # transient tunnel/device hiccups surface as runtime errors; one retry
        import time
        time.sleep(2.0)
        return run(inputs)
